# revision 39
# baseline (speedup 1.0000x reference)
"""GATv2 3-layer GNN forward on 8 Trainium2 NeuronCores (Bass/Tile).

Sharding: edges (with self-loops) sorted by dst; core c owns dst nodes
[5000c, 5000(c+1)) so all segment reductions are core-local. Node tables
for layers 2/3 are replicated via per-layer AllGather.

Per 122-node block, edges run in 128-edge tiles. The host-built block
matrix S_T (st_blk) carries, per edge column: rows 0-121 one-hot dst,
rows 122-126 the raw 5-dim input features of the edge's SOURCE node,
row 127 the edge attribute. One matmul per tile then computes the full
pre-activation for layer 1 (xr[dst] + xl[src] + ea*we, |att| prefolded
into the weight rows of the rhs), with no per-edge gather at all.

    psum_t = S_T^T @ [xr_block ; WL1f ; weaug]     (TensorE, layer 1)
    psum_t = I @ gather(table, src) + S_T^T @ [xr_block ; 0 ; weaug]
                                                   (layers 2/3; the
      gather is ONE block-sized indirect DMA, and the identity matmul
      is ONE wide matmul per edge-tile group issued FIRST so the
      per-tile one-hot matmuls accumulate on top)
    logits = sum_c sign(att_c) * prelu(psum_t, 0.2)  (ScalarE+VectorE)
    ex = exp(logits)          (softmax max-subtraction skipped: logits
                               are O(1) for this model's weight scale)

Scatter (layer 1, factored): since xl1 = ext @ WL1f is linear in the
5 raw features, sum_e ex*xl1[src] = (sum_e ex (x) ext_src) @ Wbd — the
per-tile scatter matmul only carries [ex (x) ext | ex] (48 wide), and a
single block-level matmul against the block-diagonal Wbd (with 1/|att|
refolded, i.e. the unfolded M@wl1) reconstructs the 256-wide numerator.
Layers 2/3 scatter [ex*xl | ex] directly (136 / 9 wide).

Block epilogue: h = tanh(num * (1/den) [* 1/|att|] + bias); next-layer
xl/xr via PE transpose + matmul; xl staged to DRAM and AllGathered.
Pooling: one-hot matmul on local graph ids, indirect-DMA scatter to
[512,8], AllReduce, then the tiny linear head.

Host work is index/layout preprocessing and weight fusion only.
"""
import sys

for _p in ("/opt/trn_rl_repo",):
    if _p not in sys.path:
        sys.path.insert(0, _p)

import numpy as np

N = 40000
E = 500000
B = 512
NC = 8
NPC = N // NC            # nodes per core
BLK = 122                # real nodes per 128-row block (122-126 ext, 127 ea)
NBLK = -(-NPC // BLK)    # blocks per core (41)
PADN = NBLK * 128        # padded node rows per core (5248)
HEADS = [(8, 32), (8, 16), (1, 8)]   # (H, C) per layer
DIMS = [h * c for h, c in HEADS]     # 256, 128, 8
WIDTHS = [d + h for d, (h, c) in zip(DIMS, HEADS)]  # scatter width: -, 136, 9
GBS = [4, 4, 8]          # edge tiles per elementwise batch, per layer
POOLPAD = 768
SPLIT16 = 32768          # dma_gather int16 index split point
ZLO = 122                # pad row < SPLIT16 zeroed as gather-A sentinel
ZHI = 32890              # pad row >= SPLIT16 zeroed as gather-B sentinel

_CACHE = {}


def _padrow(n):
    c, nl = np.divmod(n, NPC)
    b, r = np.divmod(nl, BLK)
    return PADN * c + 128 * b + r


def _host_preprocess(x, edge_index, edge_attr, batch):
    src = np.asarray(edge_index[0], np.int64)
    dst = np.asarray(edge_index[1], np.int64)
    ea = np.asarray(edge_attr, np.float32).reshape(-1)

    # self loops, fill_value='mean' of incoming edge_attr
    deg = np.zeros(N, np.float32)
    np.add.at(deg, dst, np.float32(1.0))
    esum = np.zeros(N, np.float32)
    np.add.at(esum, dst, ea)
    loop_attr = np.where(deg > 0, esum / np.maximum(deg, 1.0), 0.0).astype(np.float32)
    src_f = np.concatenate([src, np.arange(N, dtype=np.int64)])
    dst_f = np.concatenate([dst, np.arange(N, dtype=np.int64)])
    ea_f = np.concatenate([ea, loop_attr]).astype(np.float32)

    order = np.argsort(dst_f, kind="stable")
    src_s, dst_s, ea_s = src_f[order], dst_f[order], ea_f[order]
    src_pad = _padrow(src_s).astype(np.int32)
    assert NC * PADN == 41984 and SPLIT16 == 32768

    # raw 5-dim input features (x0, xyz, 1) of each edge's source node
    xf = np.asarray(x, np.float32)
    ext = np.concatenate([xf[:, :1], xf[:, 1:], np.ones((N, 1), np.float32)], 1)
    ext_src = ext[src_s]  # [Ef, 5]

    bounds = np.searchsorted(dst_s, np.arange(0, N + 1, 1))
    # src-range split per block: edges with src_pad < SPLIT16 occupy the
    # first kas[b] tiles (shared across cores), the rest follow, so the
    # two int16 dma_gathers cover disjoint tile ranges with no overlap
    tiles_pb, kas = [], []
    for b in range(NBLK):
        mxlo = mxhi = 0
        for c in range(NC):
            lo = bounds[min(c * NPC + b * BLK, N)]
            hi = bounds[min(c * NPC + min((b + 1) * BLK, NPC), N)]
            nlow = int((src_pad[lo:hi] < SPLIT16).sum())
            mxlo = max(mxlo, nlow)
            mxhi = max(mxhi, (hi - lo) - nlow)
        ka = -(-mxlo // 128)
        kb = -(-mxhi // 128)
        kas.append(ka)
        tiles_pb.append(ka + kb)
    T = sum(tiles_pb)

    MAXNT = max(tiles_pb)
    # block-packed S_T: per block, partition p holds its rows of all nt
    # tiles contiguously -> one line-rate DMA per block
    st_blk = np.zeros((NC, NBLK, 128, MAXNT * 128), np.float32)
    src_all = np.zeros((NC, T, 128), np.int32)
    dst_all = np.full((NC, T, 128), 200.0, np.float32)
    extE = np.zeros((NC, T, 128, 5), np.float32)
    t0 = 0
    for b in range(NBLK):
        nt = tiles_pb[b]
        ka = kas[b]
        for c in range(NC):
            lo = bounds[min(c * NPC + b * BLK, N)]
            hi = bounds[min(c * NPC + min((b + 1) * BLK, NPC), N)]
            ne = hi - lo
            sp = src_pad[lo:hi]
            # low-src edges first, then high-src starting at tile ka
            ordr = np.argsort(sp >= SPLIT16, kind="stable")
            sp = sp[ordr]
            nlow = int((sp < SPLIT16).sum())
            slot = np.empty(ne, np.int64)
            slot[:nlow] = np.arange(nlow)
            slot[nlow:] = ka * 128 + np.arange(ne - nlow)
            dl = (dst_s[lo:hi][ordr] - c * NPC - b * BLK).astype(np.int64)
            ti, pi = slot // 128, slot % 128
            st_blk[c, b, dl, ti * 128 + pi] = 1.0
            st_blk[c, b, 122:127, ti * 128 + pi] = ext_src[lo:hi][ordr]
            st_blk[c, b, 127, ti * 128 + pi] = ea_s[lo:hi][ordr]
            src_all[c, t0 + ti, pi] = sp
            dst_all[c, t0 + ti, pi] = dl.astype(np.float32)
            extE[c, t0 + ti, pi] = ext_src[lo:hi][ordr]
        t0 += nt
    src_sb = np.ascontiguousarray(src_all.transpose(0, 2, 1))
    dst_sb = np.ascontiguousarray(dst_all.transpose(0, 2, 1))
    extE_sb = np.ascontiguousarray(extE.transpose(0, 2, 1, 3)).reshape(NC, 128, T * 5)

    # dma_gather int16 indices, [16]-wrapped per gather range and replicated:
    # index i of a range starting at tile tb lives at [i % 16 (+16k),
    # tb*8 + i//16]; tiles [t0, t0+ka) gather from table[:SPLIT16], tiles
    # [t0+ka, t0+nt) from table[SPLIT16:] (pad slots point at row 0)
    idx16 = np.zeros((NC, 128, T * 8), np.int16)
    t0 = 0
    for b in range(NBLK):
        nt, ka = tiles_pb[b], kas[b]
        for c in range(NC):
            arrA = src_all[c, t0:t0 + ka, :].reshape(-1)
            assert (arrA < SPLIT16).all()
            arrB = src_all[c, t0 + ka:t0 + nt, :].reshape(-1)
            assert ((arrB == 0) | (arrB >= SPLIT16)).all()
            bb = np.where(arrB >= SPLIT16, arrB - SPLIT16, 0).astype(np.int16)
            if ka:
                idx16[c, :, t0 * 8:(t0 + ka) * 8] = np.tile(
                    arrA.astype(np.int16).reshape(ka * 8, 16).T, (8, 1))
            if nt > ka:
                idx16[c, :, (t0 + ka) * 8:(t0 + nt) * 8] = np.tile(
                    bb.reshape((nt - ka) * 8, 16).T, (8, 1))
        t0 += nt

    # pooling metadata
    batch = np.asarray(batch, np.int64)
    gbase = np.array([batch[c * NPC] for c in range(NC)], np.int64)
    batchloc = np.full((NC, 128, NBLK), 200.0, np.float32)
    for c in range(NC):
        bl = batch[c * NPC:(c + 1) * NPC] - gbase[c]
        assert bl.max() < 127, "graph span exceeds 127 per core"
        for b in range(NBLK):
            nn = min((b + 1) * BLK, NPC) - b * BLK
            batchloc[c, :nn, b] = bl[b * BLK: b * BLK + nn]
    g_rows = np.zeros((NC, 128, 1), np.int32)
    for c in range(NC):
        rows = gbase[c] + np.arange(128)
        junk = B + 64 + np.arange(128)
        g_rows[c, :, 0] = np.where(rows < B, rows, junk)
    cnt = np.bincount(batch, minlength=B).astype(np.float32)
    rcnt = (1.0 / np.maximum(cnt, 1.0)).astype(np.float32)

    return dict(tiles_pb=tiles_pb, kas=kas, T=T, MAXNT=MAXNT, st_blk=st_blk,
                idx16=idx16, src_sb=src_sb,
                dst_sb=dst_sb, extE_sb=extE_sb, batchloc=batchloc, g_rows=g_rows,
                rcnt=rcnt)


def _fuse_weights(wl, wr, we, att, H, C):
    """|att| folded into the main columns (prelu commutes with positive
    scales); sign(att) applied after prelu; 1/|att| recovers the scatter."""
    absatt = np.maximum(np.abs(att).reshape(-1), 1e-8).astype(np.float32)  # [D]
    WL = (wl * absatt[None, :]).astype(np.float32)
    WR = (wr * absatt[None, :]).astype(np.float32)
    WE = (we * absatt[None, :]).astype(np.float32)
    sgn = np.sign(att).reshape(-1).astype(np.float32)
    attrecip = (1.0 / absatt).astype(np.float32)
    return WL, WR, WE, sgn, attrecip


def _host_weights(inp):
    out = {}
    W = []
    for i, (H, C) in enumerate(HEADS, start=1):
        W.append(_fuse_weights(np.asarray(inp[f"wl{i}"], np.float32),
                               np.asarray(inp[f"wr{i}"], np.float32),
                               np.asarray(inp[f"we{i}"], np.float32),
                               np.asarray(inp[f"att{i}"], np.float32), H, C))
    # layer-1 input fusion: ext = [x0, xyz, 1]; h0 = ext @ M
    M = np.zeros((5, 7), np.float32)
    M[0, :4] = np.asarray(inp["w0"], np.float32)[0]
    M[1, 4] = M[2, 5] = M[3, 6] = 1.0
    M[4, :4] = np.asarray(inp["b0"], np.float32)
    WL1f = (M @ W[0][0]).astype(np.float32)             # [5, 256], |att|-folded
    WL1raw = (M @ np.asarray(inp["wl1"], np.float32)).astype(np.float32)  # [5,256]
    out["WR1f"] = (M @ W[0][1]).astype(np.float32)
    # xr rows 122-127 per layer: [WL1f ; weaug1], [0 ; weaug2], [0 ; weaug3]
    for i, (Hh, Cc) in enumerate(HEADS, start=1):
        Dd = Hh * Cc
        top = (np.tile(WL1f, (1, NBLK)) if i == 1
               else np.zeros((5, NBLK * Dd), np.float32))
        out[f"xrfill{i}"] = np.vstack(
            [top, np.tile(W[i - 1][2], (1, NBLK))]).astype(np.float32)
    # block-diagonal numerator reconstruction (attrecip refolded => raw wl1)
    H1, C1 = HEADS[0]
    Wbd = np.zeros((5 * H1, DIMS[0]), np.float32)
    for h in range(H1):
        Wbd[h * 5:(h + 1) * 5, h * C1:(h + 1) * C1] = WL1raw[:, h * C1:(h + 1) * C1]
    out["Wbd"] = Wbd
    for i in (2, 3):
        out[f"WL{i}"] = W[i - 1][0]
        out[f"WR{i}"] = W[i - 1][1]
    for i in (1, 2, 3):
        out[f"weaug{i}"] = np.tile(W[i - 1][2], (1, NBLK)).astype(np.float32)
        out[f"sgnB{i}"] = np.tile(W[i - 1][3][None, :], (128, 1))
        out[f"attrecip{i}"] = np.tile(W[i - 1][4][None, :], (128, 1))
        out[f"biasRep{i}"] = np.tile(np.asarray(inp[f"b{i}"], np.float32)[None, :],
                                     (128, 1))
    out["w4rep"] = np.tile(np.asarray(inp["w4"], np.float32)[:, 0][None, :], (128, 1))
    out["b4"] = float(np.asarray(inp["b4"], np.float32)[0])
    return out


def _build_x_inputs(x):
    x = np.asarray(x, np.float32)
    ext = np.concatenate([x[:, :1], x[:, 1:], np.ones((N, 1), np.float32)], 1)
    extp = np.zeros((NC * PADN, 5), np.float32)
    extp[_padrow(np.arange(N))] = ext
    xt6_own = np.ascontiguousarray(extp.reshape(NC, PADN, 5).transpose(0, 2, 1))
    return xt6_own


def _build_program(tiles_pb, kas, T):
    import contextlib
    import concourse.bass as bass
    import concourse.bacc as bacc
    import concourse.mybir as mybir
    import concourse.tile as tile

    dt = mybir.dt
    f32 = dt.float32
    bf16 = dt.bfloat16
    i32 = dt.int32
    Alu = mybir.AluOpType
    Act = mybir.ActivationFunctionType
    IOA = bass.IndirectOffsetOnAxis

    nc = bacc.Bacc("TRN2", target_bir_lowering=False, debug=False, num_devices=NC)

    ein = {}
    def EIN(name, shape, d=f32):
        ein[name] = nc.dram_tensor(name, list(shape), d, kind="ExternalInput")
        return ein[name]

    i16 = dt.int16
    MAXNT = max(tiles_pb)
    st_blk_d = EIN("st_blk", [NBLK, 128, MAXNT * 128], bf16)
    idx16_d = EIN("idx16", [128, T * 8], i16)
    dst_sb_d = EIN("dst_sb", [128, T], bf16)
    extE_d = EIN("extE", [128, T * 5], bf16)
    xt6_own_d = EIN("xt6_own", [5, PADN])
    WR1f_d = EIN("WR1f", [5, DIMS[0]])
    Wbd_d = EIN("Wbd", [40, DIMS[0]], bf16)
    WL2_d = EIN("WL2", [DIMS[0], DIMS[1]], bf16)
    WR2_d = EIN("WR2", [DIMS[0], DIMS[1]], bf16)
    WL3_d = EIN("WL3", [DIMS[1], DIMS[2]], bf16)
    WR3_d = EIN("WR3", [DIMS[1], DIMS[2]], bf16)
    xrfill_d = [EIN(f"xrfill{i}", [6, NBLK * DIMS[i - 1]], bf16) for i in (1, 2, 3)]
    sgnB_d = [EIN(f"sgnB{i}", [128, DIMS[i - 1]], bf16) for i in (1, 2, 3)]
    attrecip_d = [EIN(f"attrecip{i}", [128, DIMS[i - 1]]) for i in (2, 3)]
    biasRep_d = [EIN(f"biasRep{i}", [128, DIMS[i - 1]]) for i in (1, 2, 3)]
    iota_d = EIN("iota_row", [128, 128], bf16)
    ident_d = EIN("ident", [128, 128], bf16)
    batchloc_d = EIN("batchloc", [128, NBLK], bf16)
    g_rows_d = EIN("g_rows", [128, 1], i32)
    rcnt_d = EIN("rcnt", [128, 4])
    w4rep_d = EIN("w4rep", [128, 8])
    b4_d = EIN("b4v", [128, 1])

    out_d = nc.dram_tensor("out", [B, 1], f32, kind="ExternalOutput")

    # table/stage row width padded to 128 for layer 3 (dma_gather needs
    # 256-byte rows); cols 8:128 are never read
    TW = [None, DIMS[1], 128]
    tables = [None,
              nc.dram_tensor("table2", [NC * PADN, TW[1]], bf16,
                             addr_space="Shared"),
              nc.dram_tensor("table3", [NC * PADN, TW[2]], bf16,
                             addr_space="Shared")]
    stages = [nc.dram_tensor("stage2", [PADN, TW[1]], bf16),
              nc.dram_tensor("stage3", [PADN, TW[2]], bf16)]
    pool_full = nc.dram_tensor("pool_full", [POOLPAD, 8], f32)
    pool_red = nc.dram_tensor("pool_red", [B, 8], f32, addr_space="Shared")

    with tile.TileContext(nc) as tc:
        ctx = contextlib.ExitStack()
        with ctx:
            consts = ctx.enter_context(tc.tile_pool(name="consts", bufs=1))
            meta = ctx.enter_context(tc.tile_pool(name="meta", bufs=1))
            xrp = ctx.enter_context(tc.tile_pool(name="xrp", bufs=1))
            stp = ctx.enter_context(tc.tile_pool(name="stp", bufs=3))
            gp = ctx.enter_context(tc.tile_pool(name="gp", bufs=2))
            sp = ctx.enter_context(tc.tile_pool(name="sp", bufs=4))
            ep = ctx.enter_context(tc.tile_pool(name="ep", bufs=2))
            pst = ctx.enter_context(tc.tile_pool(name="psum_t", bufs=2, space="PSUM"))
            psb = ctx.enter_context(tc.tile_pool(name="psum_blk", bufs=2, space="PSUM"))
            pse = ctx.enter_context(tc.tile_pool(name="psum_epi", bufs=1, space="PSUM"))
            chp = ctx.enter_context(tc.tile_pool(name="chunk", bufs=2))

            def load_const(dram, shape, d=f32):
                t = consts.tile(list(shape), d, tag=dram.name + "_c")
                nc.sync.dma_start(t[:], dram[:])
                return t
            iota_t = load_const(iota_d, [128, 128], bf16)
            ident_t = load_const(ident_d, [128, 128], bf16)
            WR1f_t = load_const(WR1f_d, [5, DIMS[0]])
            Wbd_t = load_const(Wbd_d, [40, DIMS[0]], bf16)
            WL2_t = [consts.tile([128, DIMS[1]], bf16, tag=f"wl2_{k}", name=f"wl2_{k}")
                     for k in range(2)]
            WR2_t = [consts.tile([128, DIMS[1]], bf16, tag=f"wr2_{k}", name=f"wr2_{k}")
                     for k in range(2)]
            for k in range(2):
                nc.sync.dma_start(WL2_t[k][:], WL2_d[k * 128:(k + 1) * 128, :])
                nc.sync.dma_start(WR2_t[k][:], WR2_d[k * 128:(k + 1) * 128, :])
            WL3_t = load_const(WL3_d, [128, DIMS[2]], bf16)
            WR3_t = load_const(WR3_d, [128, DIMS[2]], bf16)
            sgnB_t = [load_const(sgnB_d[i], [128, DIMS[i]], bf16) for i in range(3)]
            attrecip_t = [None,
                          load_const(attrecip_d[0], [128, DIMS[1]]),
                          load_const(attrecip_d[1], [128, DIMS[2]])]
            biasRep_t = [load_const(biasRep_d[i], [128, DIMS[i]]) for i in range(3)]
            batchloc_t = load_const(batchloc_d, [128, NBLK], bf16)
            g_rows_t = load_const(g_rows_d, [128, 1], i32)
            rcnt_t = load_const(rcnt_d, [128, 4])
            w4rep_t = load_const(w4rep_d, [128, 8])
            b4_t = load_const(b4_d, [128, 1])
            idx_t = meta.tile([128, T * 8], i16)
            nc.sync.dma_start(idx_t[:], idx16_d[:])
            dst_t = meta.tile([128, T], bf16)
            nc.sync.dma_start(dst_t[:], dst_sb_d[:])
            extE_t = meta.tile([128, T, 5], bf16)
            nc.sync.dma_start(extE_t[:].rearrange("p t f -> p (t f)"), extE_d[:])

            xr_t = [xrp.tile([128, NBLK * DIMS[i]], bf16, tag=f"xr{i}", name=f"xr{i}")
                    for i in range(3)]
            # rows 122-127 per layer: [WL1f ; weaug1] for layer 1,
            # [zeros ; weaug] for layers 2/3
            for i in range(3):
                nc.sync.dma_start(xr_t[i][122:128, :], xrfill_d[i][:])

            # ---- preamble: own xr1 blocks (f32 math, bf16 out) ----
            CH = 16
            for ch in range(-(-NBLK // CH)):
                j0, j1 = ch * CH, min((ch + 1) * CH, NBLK)
                xchunk = chp.tile([5, CH * 128], f32, tag="xchunk")
                nc.sync.dma_start(xchunk[:, :(j1 - j0) * 128],
                                  xt6_own_d[:, j0 * 128:j1 * 128])
                for j in range(j1 - j0):
                    b = j0 + j
                    pt = pse.tile([128, DIMS[0]], f32, tag="epi_ps", space="PSUM")
                    nc.tensor.matmul(pt[:], lhsT=xchunk[:, j * 128:(j + 1) * 128],
                                     rhs=WR1f_t[:], start=True, stop=True)
                    D0 = DIMS[0]
                    nc.vector.tensor_copy(xr_t[0][0:122, b * D0:(b + 1) * D0],
                                          pt[0:122, :])

            # ---- layers ----
            pool_ps = psb.tile([128, 8], f32, tag="pool_ps", space="PSUM", bufs=1)
            for li in range(3):
                H, C = HEADS[li]
                D = DIMS[li]
                GB = GBS[li]
                W = 48 if li == 0 else WIDTHS[li]
                table = tables[li]
                is_last = li == 2

                if li > 0:
                    nc.gpsimd.collective_compute(
                        "AllGather", Alu.bypass,
                        replica_groups=[list(range(NC))],
                        ins=[stages[li - 1].ap().opt()],
                        outs=[table.ap().opt()],
                    )

                t0 = 0
                for b in range(NBLK):
                    nt = tiles_pb[b]
                    ka = kas[b]
                    pblk = psb.tile([128, W], f32, tag="blk_ps", space="PSUM")
                    sblk = stp.tile([128, MAXNT * 128], bf16, tag="st_blk", bufs=3)
                    nc.sync.dma_start(sblk[:, :nt * 128], st_blk_d[b, :, :nt * 128])
                    if li > 0:
                        gblk = gp.tile([128, MAXNT, TW[li]], bf16, tag="g_blk",
                                       name="g_blk", bufs=3)
                        if ka:
                            nc.gpsimd.dma_gather(
                                gblk[:, 0:ka, :], table[0:SPLIT16, :],
                                idx_t[:, t0 * 8:(t0 + ka) * 8], ka * 128,
                                ka * 128, TW[li], single_packet=False)
                        if nt > ka:
                            nc.gpsimd.dma_gather(
                                gblk[:, ka:nt, :], table[SPLIT16:NC * PADN, :],
                                idx_t[:, (t0 + ka) * 8:(t0 + nt) * 8],
                                (nt - ka) * 128, (nt - ka) * 128, TW[li],
                                single_packet=False)
                    # pass 1: per-tile pre-activations + per-edge scatter rows
                    # into block-sized buffers (keeps the PE MM1 stream dense)
                    ytb = gp.tile([128, MAXNT, max(W, 48)], bf16, tag="y_blk",
                                  name="y_blk", bufs=2)
                    smb = stp.tile([128, MAXNT * 128], bf16, tag="s_blk", bufs=2)
                    for g0 in range(0, nt, GB):
                        gs = min(GB, nt - g0)
                        ptile = pst.tile([128, GB, D], f32, tag="t_ps",
                                         name=f"t_ps{li}", space="PSUM")
                        if li > 0:
                            # wide gather-add FIRST (opens the psum group for
                            # the whole bank), then per-tile one-hot matmuls
                            # accumulate on top
                            nc.tensor.matmul(
                                ptile[:, 0:gs, 0:D], lhsT=ident_t[:],
                                rhs=gblk[:, g0:g0 + gs, 0:D],
                                start=True, stop=False, skip_group_check=True)
                        for i in range(gs):
                            nc.tensor.matmul(
                                ptile[:, i, 0:D],
                                lhsT=sblk[:, (g0 + i) * 128:(g0 + i + 1) * 128],
                                rhs=xr_t[li][:, b * D:(b + 1) * D],
                                start=(li == 0),
                                stop=(li == 0) or (i == gs - 1),
                                skip_group_check=(li > 0))
                        # logits = sum_c sgn * prelu(t_s, 0.2); ex = exp(logits)
                        u = sp.tile([128, GB * D], bf16, tag="u_t")
                        nc.scalar.activation(
                            u[:, :gs * D].rearrange("p (g d) -> p g d", g=gs),
                            ptile[:, 0:gs, 0:D], Act.Prelu, alpha=0.2)
                        v = sp.tile([128, GB * D], bf16, tag="v_t")
                        veng = nc.gpsimd if li == 0 else nc.vector
                        veng.tensor_tensor(
                            out=v[:, :gs * D].rearrange("p (g d) -> p g d", g=gs),
                            in0=u[:, :gs * D].rearrange("p (g d) -> p g d", g=gs),
                            in1=sgnB_t[li][:].unsqueeze(1).to_broadcast([128, gs, D]),
                            op=Alu.mult)
                        lg = sp.tile([128, GB * H], f32, tag="lg")
                        nc.vector.tensor_reduce(
                            out=lg[:, :gs * H].rearrange("p (g h) -> p g h", g=gs),
                            in_=v[:, :gs * D].rearrange("p (g h c) -> p g h c",
                                                        g=gs, h=H),
                            axis=mybir.AxisListType.X, op=Alu.add)
                        yt = ytb[:, g0:g0 + gs, 0:W]
                        nc.scalar.activation(
                            yt[:, :, W - H:W], lg[:, :gs * H].rearrange(
                                "p (g h) -> p g h", g=gs), Act.Exp)
                        if li == 0:
                            nc.vector.tensor_tensor(
                                out=yt[:, :, 0:40].rearrange(
                                    "p g (h f) -> p g h f", h=8),
                                in0=yt[:, :, 40:48].unsqueeze(3)
                                    .to_broadcast([128, gs, 8, 5]),
                                in1=extE_t[:, t0 + g0:t0 + g0 + gs, :].unsqueeze(2)
                                    .to_broadcast([128, gs, 8, 5]),
                                op=Alu.mult)
                        else:
                            nc.vector.tensor_tensor(
                                out=yt[:, :, 0:D].rearrange(
                                    "p g (h c) -> p g h c", h=H),
                                in0=gblk[:, g0:g0 + gs, 0:D].rearrange(
                                    "p g (h c) -> p g h c", h=H),
                                in1=yt[:, :, D:W].unsqueeze(3)
                                    .to_broadcast([128, gs, H, C]),
                                op=Alu.mult)
                        nc.vector.tensor_tensor(
                            out=smb[:, g0 * 128:(g0 + gs) * 128].rearrange(
                                "p (g n) -> p g n", g=gs),
                            in0=dst_t[:, t0 + g0:t0 + g0 + gs].unsqueeze(2)
                                .to_broadcast([128, gs, 128]),
                            in1=iota_t[:].unsqueeze(1).to_broadcast([128, gs, 128]),
                            op=Alu.is_equal)
                    # pass 2: all scatter matmuls back-to-back
                    for t in range(nt):
                        nc.tensor.matmul(
                            pblk[:], lhsT=smb[:, t * 128:(t + 1) * 128],
                            rhs=ytb[:, t, 0:W],
                            start=(t == 0), stop=(t == nt - 1))
                    t0 += nt

                    # ---- block epilogue ----
                    den = sp.tile([128, H], f32, tag="den")
                    nc.vector.tensor_scalar_add(den[:], pblk[:, W - H:W], 1e-30)
                    rden = sp.tile([128, H], f32, tag="rden")
                    nc.vector.reciprocal(rden[:], den[:])
                    if li == 0:
                        # reconstruct numerator: (scat @ Wbd), attrecip refolded
                        scat_sb = ep.tile([128, 48], bf16, tag="scat_sb")
                        nc.vector.tensor_copy(scat_sb[:], pblk[:, 0:48])
                        tps = pse.tile([128, 128], bf16, tag="epi_ps", space="PSUM")
                        nc.tensor.transpose(tps[0:48, :], scat_sb[:], ident_t[:])
                        scatT = ep.tile([48, 128], bf16, tag="scatT")
                        nc.vector.tensor_copy(scatT[:], tps[0:48, :])
                        pnum = pse.tile([128, DIMS[0]], f32, tag="epi_ps",
                                        space="PSUM")
                        nc.tensor.matmul(pnum[:], lhsT=scatT[0:40, :], rhs=Wbd_t[:],
                                         start=True, stop=True)
                        hr = ep.tile([128, D], f32, tag="hr")
                        nc.vector.tensor_tensor(
                            out=hr[:].rearrange("p (h c) -> p h c", h=H),
                            in0=pnum[:].rearrange("p (h c) -> p h c", h=H),
                            in1=rden[:].unsqueeze(2).to_broadcast([128, H, C]),
                            op=Alu.mult)
                    else:
                        hr = ep.tile([128, D], f32, tag="hr")
                        nc.vector.tensor_tensor(
                            out=hr[:].rearrange("p (h c) -> p h c", h=H),
                            in0=pblk[:, 0:D].rearrange("p (h c) -> p h c", h=H),
                            in1=rden[:].unsqueeze(2).to_broadcast([128, H, C]),
                            op=Alu.mult)
                        nc.vector.tensor_tensor(out=hr[:], in0=hr[:],
                                                in1=attrecip_t[li][:], op=Alu.mult)
                    nc.vector.tensor_tensor(out=hr[:], in0=hr[:],
                                            in1=biasRep_t[li][:], op=Alu.add)
                    h = ep.tile([128, D], bf16, tag="h_blk")
                    nc.scalar.activation(h[:], hr[:], Act.Tanh)

                    if not is_last:
                        D2 = DIMS[li + 1]
                        WLn = [WL2_t[0], WL2_t[1]] if li == 0 else [WL3_t]
                        WRn = [WR2_t[0], WR2_t[1]] if li == 0 else [WR3_t]
                        nk = D // 128
                        hT = []
                        for k in range(nk):
                            tp = pse.tile([128, 128], bf16, tag="epi_ps",
                                          space="PSUM")
                            nc.tensor.transpose(tp[:], h[:, k * 128:(k + 1) * 128],
                                                ident_t[:])
                            hTk = ep.tile([128, 128], bf16, tag=f"hT{k}")
                            nc.vector.tensor_copy(hTk[:], tp[:])
                            hT.append(hTk)
                        pxl = pse.tile([128, D2], f32, tag="epi_ps", space="PSUM")
                        for k in range(nk):
                            nc.tensor.matmul(pxl[:], lhsT=hT[k][:], rhs=WLn[k][:],
                                             start=(k == 0), stop=(k == nk - 1))
                        xlout = ep.tile([128, D2], bf16, tag="xlout")
                        nc.vector.tensor_copy(xlout[:], pxl[:])
                        nc.sync.dma_start(
                            stages[li][b * 128:(b + 1) * 128, 0:D2], xlout[:])
                        pxr = pse.tile([128, D2], f32, tag="epi_ps", space="PSUM")
                        for k in range(nk):
                            nc.tensor.matmul(pxr[:], lhsT=hT[k][:], rhs=WRn[k][:],
                                             start=(k == 0), stop=(k == nk - 1))
                        nc.vector.tensor_copy(
                            xr_t[li + 1][0:122, b * D2:(b + 1) * D2], pxr[0:122, :])
                    else:
                        Sg = stp.tile([128, 128], bf16, tag="sg_tile")
                        nc.vector.tensor_tensor(
                            out=Sg[:],
                            in0=batchloc_t[:, b:b + 1].to_broadcast([128, 128]),
                            in1=iota_t[:], op=Alu.is_equal)
                        nc.tensor.matmul(pool_ps[:], lhsT=Sg[:], rhs=h[:],
                                         start=(b == 0), stop=(b == NBLK - 1))

            # ---- pooling + head ----
            pool_sb = ep.tile([128, 8], f32, tag="pool_sb")
            nc.vector.tensor_copy(pool_sb[:], pool_ps[:])
            zero8 = consts.tile([128, 8], f32, tag="zero8")
            nc.gpsimd.memset(zero8[:], 0.0)
            for i in range(POOLPAD // 128):
                nc.sync.dma_start(pool_full[i * 128:(i + 1) * 128, :], zero8[:])
            nc.gpsimd.indirect_dma_start(
                out=pool_full[:], out_offset=IOA(ap=g_rows_t[:, :1], axis=0),
                in_=pool_sb[:], in_offset=None)
            nc.gpsimd.collective_compute(
                "AllReduce", Alu.add, replica_groups=[list(range(NC))],
                ins=[pool_full.ap()[0:B, :].opt()], outs=[pool_red.ap().opt()])
            for i in range(B // 128):
                pt = ep.tile([128, 8], f32, tag="head_in")
                nc.sync.dma_start(pt[:], pool_red[i * 128:(i + 1) * 128, :])
                pw = ep.tile([128, 8], f32, tag="head_w")
                nc.vector.tensor_tensor(out=pw[:], in0=pt[:], in1=w4rep_t[:],
                                        op=Alu.mult)
                hred = ep.tile([128, 1], f32, tag="head_red")
                nc.vector.tensor_reduce(out=hred[:], in_=pw[:],
                                        axis=mybir.AxisListType.X, op=Alu.add)
                nc.vector.tensor_tensor(out=hred[:], in0=hred[:],
                                        in1=rcnt_t[:, i:i + 1], op=Alu.mult)
                nc.vector.tensor_tensor(out=hred[:], in0=hred[:], in1=b4_t[:],
                                        op=Alu.add)
                nc.sync.dma_start(out_d[i * 128:(i + 1) * 128, :], hred[:])

    nc.compile()
    return nc


def _get_program(inputs):
    pre = _host_preprocess(inputs["x"], inputs["edge_index"], inputs["edge_attr"],
                           inputs["batch"])
    key = (tuple(pre["tiles_pb"]), tuple(pre["kas"]))
    if key not in _CACHE:
        _CACHE[key] = _build_program(pre["tiles_pb"], pre["kas"], pre["T"])
    return _CACHE[key], pre


def _make_in_maps(inputs, pre):
    import ml_dtypes
    bf16 = ml_dtypes.bfloat16
    wts = _host_weights(inputs)
    xt6_own = _build_x_inputs(inputs["x"])
    iota = np.tile(np.arange(128, dtype=np.float32), (128, 1))
    ident = np.eye(128, dtype=np.float32)
    in_maps = []
    for c in range(NC):
        m = dict(
            st_blk=pre["st_blk"][c].astype(bf16),
            idx16=pre["idx16"][c],
            dst_sb=pre["dst_sb"][c].astype(bf16),
            extE=pre["extE_sb"][c].astype(bf16),
            xt6_own=xt6_own[c],
            WR1f=wts["WR1f"], Wbd=wts["Wbd"].astype(bf16),
            WL2=wts["WL2"].astype(bf16), WR2=wts["WR2"].astype(bf16),
            WL3=wts["WL3"].astype(bf16), WR3=wts["WR3"].astype(bf16),
            iota_row=iota.astype(bf16), ident=ident.astype(bf16),
            batchloc=pre["batchloc"][c].astype(bf16), g_rows=pre["g_rows"][c],
            rcnt=np.ascontiguousarray(pre["rcnt"].reshape(4, 128).T),
            w4rep=wts["w4rep"], b4v=np.full((128, 1), wts["b4"], np.float32),
        )
        for i in (1, 2, 3):
            m[f"xrfill{i}"] = wts[f"xrfill{i}"].astype(bf16)
            m[f"sgnB{i}"] = wts[f"sgnB{i}"].astype(bf16)
            m[f"biasRep{i}"] = wts[f"biasRep{i}"]
            if i > 1:
                m[f"attrecip{i}"] = wts[f"attrecip{i}"]
        in_maps.append(m)
    return in_maps


def kernel(**inputs):
    from concourse.bass_utils import run_bass_kernel_spmd
    nc, pre = _get_program(inputs)
    in_maps = _make_in_maps(inputs, pre)
    res = run_bass_kernel_spmd(nc, in_maps, core_ids=list(range(NC)))
    return np.asarray(res.results[0]["out"], np.float32)


# revision 40
# speedup vs baseline: 1.2089x; 1.2089x over previous
"""GATv2 3-layer GNN forward on 8 Trainium2 NeuronCores (Bass/Tile).

Sharding: edges (with self-loops) sorted by dst; core c owns dst nodes
[5000c, 5000(c+1)) so all segment reductions are core-local. Node tables
for layers 2/3 are replicated via per-layer AllGather.

Per 122-node block, edges run in 128-edge tiles. The host-built block
matrix S_T (st_blk) carries, per edge column: rows 0-121 one-hot dst,
rows 122-126 the raw 5-dim input features of the edge's SOURCE node,
row 127 the edge attribute. One matmul per tile then computes the full
pre-activation for layer 1 (xr[dst] + xl[src] + ea*we, |att| prefolded
into the weight rows of the rhs), with no per-edge gather at all.

    psum_t = S_T^T @ [xr_block ; WL1f ; weaug]     (TensorE, layer 1)
    psum_t = I @ gather(table, src) + S_T^T @ [xr_block ; 0 ; weaug]
                                                   (layers 2/3; the
      gather is ONE block-sized indirect DMA, and the identity matmul
      is ONE wide matmul per edge-tile group issued FIRST so the
      per-tile one-hot matmuls accumulate on top)
    logits = sum_c sign(att_c) * prelu(psum_t, 0.2)  (ScalarE+VectorE)
    ex = exp(logits)          (softmax max-subtraction skipped: logits
                               are O(1) for this model's weight scale)

Scatter (layer 1, factored): since xl1 = ext @ WL1f is linear in the
5 raw features, sum_e ex*xl1[src] = (sum_e ex (x) ext_src) @ Wbd — the
per-tile scatter matmul only carries [ex (x) ext | ex] (48 wide), and a
single block-level matmul against the block-diagonal Wbd (with 1/|att|
refolded, i.e. the unfolded M@wl1) reconstructs the 256-wide numerator.
Layers 2/3 scatter [ex*xl | ex] directly (136 / 9 wide).

Block epilogue: h = tanh(num * (1/den) [* 1/|att|] + bias); next-layer
xl/xr via PE transpose + matmul; xl staged to DRAM and AllGathered.
Pooling: one-hot matmul on local graph ids, indirect-DMA scatter to
[512,8], AllReduce, then the tiny linear head.

Host work is index/layout preprocessing and weight fusion only.
"""
import sys

for _p in ("/opt/trn_rl_repo",):
    if _p not in sys.path:
        sys.path.insert(0, _p)

import numpy as np

N = 40000
E = 500000
B = 512
NC = 8
NPC = N // NC            # nodes per core
BLK = 122                # real nodes per 128-row block (122-126 ext, 127 ea)
NBLK = -(-NPC // BLK)    # blocks per core (41)
PADN = NBLK * 128        # padded node rows per core (5248)
HEADS = [(8, 32), (8, 16), (1, 8)]   # (H, C) per layer
DIMS = [h * c for h, c in HEADS]     # 256, 128, 8
WIDTHS = [d + h for d, (h, c) in zip(DIMS, HEADS)]  # scatter width: -, 136, 9
GBS = [4, 4, 8]          # edge tiles per elementwise batch, per layer
POOLPAD = 768
SPLIT16 = 32768          # dma_gather int16 index split point
ZLO = 122                # pad row < SPLIT16 zeroed as gather-A sentinel
ZHI = 32890              # pad row >= SPLIT16 zeroed as gather-B sentinel

_CACHE = {}


def _padrow(n):
    c, nl = np.divmod(n, NPC)
    b, r = np.divmod(nl, BLK)
    return PADN * c + 128 * b + r


def _host_preprocess(x, edge_index, edge_attr, batch):
    src = np.asarray(edge_index[0], np.int64)
    dst = np.asarray(edge_index[1], np.int64)
    ea = np.asarray(edge_attr, np.float32).reshape(-1)

    # self loops, fill_value='mean' of incoming edge_attr
    deg = np.zeros(N, np.float32)
    np.add.at(deg, dst, np.float32(1.0))
    esum = np.zeros(N, np.float32)
    np.add.at(esum, dst, ea)
    loop_attr = np.where(deg > 0, esum / np.maximum(deg, 1.0), 0.0).astype(np.float32)
    src_f = np.concatenate([src, np.arange(N, dtype=np.int64)])
    dst_f = np.concatenate([dst, np.arange(N, dtype=np.int64)])
    ea_f = np.concatenate([ea, loop_attr]).astype(np.float32)

    order = np.argsort(dst_f, kind="stable")
    src_s, dst_s, ea_s = src_f[order], dst_f[order], ea_f[order]
    src_pad = _padrow(src_s).astype(np.int32)
    assert NC * PADN == 41984 and SPLIT16 == 32768

    # raw 5-dim input features (x0, xyz, 1) of each edge's source node
    xf = np.asarray(x, np.float32)
    ext = np.concatenate([xf[:, :1], xf[:, 1:], np.ones((N, 1), np.float32)], 1)
    ext_src = ext[src_s]  # [Ef, 5]

    bounds = np.searchsorted(dst_s, np.arange(0, N + 1, 1))
    # src-range split per block: edges with src_pad < SPLIT16 occupy the
    # first kas[b] tiles (shared across cores), the rest follow, so the
    # two int16 dma_gathers cover disjoint tile ranges with no overlap
    tiles_pb, kas = [], []
    for b in range(NBLK):
        mxlo = mxhi = 0
        for c in range(NC):
            lo = bounds[min(c * NPC + b * BLK, N)]
            hi = bounds[min(c * NPC + min((b + 1) * BLK, NPC), N)]
            nlow = int((src_pad[lo:hi] < SPLIT16).sum())
            mxlo = max(mxlo, nlow)
            mxhi = max(mxhi, (hi - lo) - nlow)
        ka = -(-mxlo // 128)
        kb = -(-mxhi // 128)
        kas.append(ka)
        tiles_pb.append(ka + kb)
    T = sum(tiles_pb)

    MAXNT = max(tiles_pb)
    # block-packed S_T: per block, partition p holds its rows of all nt
    # tiles contiguously -> one line-rate DMA per block
    st_blk = np.zeros((NC, NBLK, 128, MAXNT * 128), np.float32)
    src_all = np.zeros((NC, T, 128), np.int32)
    dst_all = np.full((NC, T, 128), 200.0, np.float32)
    extE = np.zeros((NC, T, 128, 5), np.float32)
    t0 = 0
    for b in range(NBLK):
        nt = tiles_pb[b]
        ka = kas[b]
        for c in range(NC):
            lo = bounds[min(c * NPC + b * BLK, N)]
            hi = bounds[min(c * NPC + min((b + 1) * BLK, NPC), N)]
            ne = hi - lo
            sp = src_pad[lo:hi]
            # low-src edges first, then high-src starting at tile ka
            ordr = np.argsort(sp >= SPLIT16, kind="stable")
            sp = sp[ordr]
            nlow = int((sp < SPLIT16).sum())
            slot = np.empty(ne, np.int64)
            slot[:nlow] = np.arange(nlow)
            slot[nlow:] = ka * 128 + np.arange(ne - nlow)
            dl = (dst_s[lo:hi][ordr] - c * NPC - b * BLK).astype(np.int64)
            ti, pi = slot // 128, slot % 128
            st_blk[c, b, dl, ti * 128 + pi] = 1.0
            st_blk[c, b, 122:127, ti * 128 + pi] = ext_src[lo:hi][ordr]
            st_blk[c, b, 127, ti * 128 + pi] = ea_s[lo:hi][ordr]
            src_all[c, t0 + ti, pi] = sp
            dst_all[c, t0 + ti, pi] = dl.astype(np.float32)
            extE[c, t0 + ti, pi] = ext_src[lo:hi][ordr]
        t0 += nt
    src_sb = np.ascontiguousarray(src_all.transpose(0, 2, 1))
    dst_sb = np.ascontiguousarray(dst_all.transpose(0, 2, 1))
    extE_sb = np.ascontiguousarray(extE.transpose(0, 2, 1, 3)).reshape(NC, 128, T * 5)

    # dma_gather int16 indices, [16]-wrapped per gather range and replicated:
    # index i of a range starting at tile tb lives at [i % 16 (+16k),
    # tb*8 + i//16]; tiles [t0, t0+ka) gather from table[:SPLIT16], tiles
    # [t0+ka, t0+nt) from table[SPLIT16:] (pad slots point at row 0)
    idx16 = np.zeros((NC, 128, T * 8), np.int16)
    t0 = 0
    for b in range(NBLK):
        nt, ka = tiles_pb[b], kas[b]
        for c in range(NC):
            arrA = src_all[c, t0:t0 + ka, :].reshape(-1)
            assert (arrA < SPLIT16).all()
            arrB = src_all[c, t0 + ka:t0 + nt, :].reshape(-1)
            assert ((arrB == 0) | (arrB >= SPLIT16)).all()
            bb = np.where(arrB >= SPLIT16, arrB - SPLIT16, 0).astype(np.int16)
            if ka:
                idx16[c, :, t0 * 8:(t0 + ka) * 8] = np.tile(
                    arrA.astype(np.int16).reshape(ka * 8, 16).T, (8, 1))
            if nt > ka:
                idx16[c, :, (t0 + ka) * 8:(t0 + nt) * 8] = np.tile(
                    bb.reshape((nt - ka) * 8, 16).T, (8, 1))
        t0 += nt

    # pooling metadata
    batch = np.asarray(batch, np.int64)
    gbase = np.array([batch[c * NPC] for c in range(NC)], np.int64)
    batchloc = np.full((NC, 128, NBLK), 200.0, np.float32)
    for c in range(NC):
        bl = batch[c * NPC:(c + 1) * NPC] - gbase[c]
        assert bl.max() < 127, "graph span exceeds 127 per core"
        for b in range(NBLK):
            nn = min((b + 1) * BLK, NPC) - b * BLK
            batchloc[c, :nn, b] = bl[b * BLK: b * BLK + nn]
    g_rows = np.zeros((NC, 128, 1), np.int32)
    for c in range(NC):
        rows = gbase[c] + np.arange(128)
        junk = B + 64 + np.arange(128)
        g_rows[c, :, 0] = np.where(rows < B, rows, junk)
    cnt = np.bincount(batch, minlength=B).astype(np.float32)
    rcnt = (1.0 / np.maximum(cnt, 1.0)).astype(np.float32)

    return dict(tiles_pb=tiles_pb, kas=kas, T=T, MAXNT=MAXNT, st_blk=st_blk,
                idx16=idx16, src_sb=src_sb,
                dst_sb=dst_sb, extE_sb=extE_sb, batchloc=batchloc, g_rows=g_rows,
                rcnt=rcnt)


def _fuse_weights(wl, wr, we, att, H, C):
    """|att| folded into the main columns (prelu commutes with positive
    scales); sign(att) applied after prelu; 1/|att| recovers the scatter."""
    absatt = np.maximum(np.abs(att).reshape(-1), 1e-8).astype(np.float32)  # [D]
    WL = (wl * absatt[None, :]).astype(np.float32)
    WR = (wr * absatt[None, :]).astype(np.float32)
    WE = (we * absatt[None, :]).astype(np.float32)
    sgn = np.sign(att).reshape(-1).astype(np.float32)
    attrecip = (1.0 / absatt).astype(np.float32)
    return WL, WR, WE, sgn, attrecip


def _host_weights(inp):
    out = {}
    W = []
    for i, (H, C) in enumerate(HEADS, start=1):
        W.append(_fuse_weights(np.asarray(inp[f"wl{i}"], np.float32),
                               np.asarray(inp[f"wr{i}"], np.float32),
                               np.asarray(inp[f"we{i}"], np.float32),
                               np.asarray(inp[f"att{i}"], np.float32), H, C))
    # layer-1 input fusion: ext = [x0, xyz, 1]; h0 = ext @ M
    M = np.zeros((5, 7), np.float32)
    M[0, :4] = np.asarray(inp["w0"], np.float32)[0]
    M[1, 4] = M[2, 5] = M[3, 6] = 1.0
    M[4, :4] = np.asarray(inp["b0"], np.float32)
    WL1f = (M @ W[0][0]).astype(np.float32)             # [5, 256], |att|-folded
    WL1raw = (M @ np.asarray(inp["wl1"], np.float32)).astype(np.float32)  # [5,256]
    out["WR1f"] = (M @ W[0][1]).astype(np.float32)
    # xr rows 122-127 per layer: [WL1f ; weaug1], [0 ; weaug2], [0 ; weaug3]
    for i, (Hh, Cc) in enumerate(HEADS, start=1):
        Dd = Hh * Cc
        top = (np.tile(WL1f, (1, NBLK)) if i == 1
               else np.zeros((5, NBLK * Dd), np.float32))
        out[f"xrfill{i}"] = np.vstack(
            [top, np.tile(W[i - 1][2], (1, NBLK))]).astype(np.float32)
    # block-diagonal numerator reconstruction (attrecip refolded => raw wl1)
    H1, C1 = HEADS[0]
    Wbd = np.zeros((5 * H1, DIMS[0]), np.float32)
    for h in range(H1):
        Wbd[h * 5:(h + 1) * 5, h * C1:(h + 1) * C1] = WL1raw[:, h * C1:(h + 1) * C1]
    out["Wbd"] = Wbd
    for i in (2, 3):
        out[f"WL{i}"] = W[i - 1][0]
        out[f"WR{i}"] = W[i - 1][1]
    for i in (1, 2, 3):
        out[f"weaug{i}"] = np.tile(W[i - 1][2], (1, NBLK)).astype(np.float32)
        out[f"sgnB{i}"] = np.tile(W[i - 1][3][None, :], (128, 1))
        out[f"attrecip{i}"] = np.tile(W[i - 1][4][None, :], (128, 1))
        out[f"biasRep{i}"] = np.tile(np.asarray(inp[f"b{i}"], np.float32)[None, :],
                                     (128, 1))
    out["w4rep"] = np.tile(np.asarray(inp["w4"], np.float32)[:, 0][None, :], (128, 1))
    out["b4"] = float(np.asarray(inp["b4"], np.float32)[0])
    return out


def _build_x_inputs(x):
    x = np.asarray(x, np.float32)
    ext = np.concatenate([x[:, :1], x[:, 1:], np.ones((N, 1), np.float32)], 1)
    extp = np.zeros((NC * PADN, 5), np.float32)
    extp[_padrow(np.arange(N))] = ext
    xt6_own = np.ascontiguousarray(extp.reshape(NC, PADN, 5).transpose(0, 2, 1))
    return xt6_own


def _build_program(tiles_pb, kas, T):
    import contextlib
    import concourse.bass as bass
    import concourse.bacc as bacc
    import concourse.mybir as mybir
    import concourse.tile as tile

    dt = mybir.dt
    f32 = dt.float32
    bf16 = dt.bfloat16
    i32 = dt.int32
    Alu = mybir.AluOpType
    Act = mybir.ActivationFunctionType
    IOA = bass.IndirectOffsetOnAxis

    nc = bacc.Bacc("TRN2", target_bir_lowering=False, debug=False, num_devices=NC)

    ein = {}
    def EIN(name, shape, d=f32):
        ein[name] = nc.dram_tensor(name, list(shape), d, kind="ExternalInput")
        return ein[name]

    i16 = dt.int16
    MAXNT = max(tiles_pb)
    st_blk_d = EIN("st_blk", [NBLK, 128, MAXNT * 128], bf16)
    idx16_d = EIN("idx16", [128, T * 8], i16)
    dst_sb_d = EIN("dst_sb", [128, T], bf16)
    extE_d = EIN("extE", [128, T * 5], bf16)
    xt6_own_d = EIN("xt6_own", [5, PADN])
    WR1f_d = EIN("WR1f", [5, DIMS[0]])
    Wbd_d = EIN("Wbd", [40, DIMS[0]], bf16)
    WL2_d = EIN("WL2", [DIMS[0], DIMS[1]], bf16)
    WR2_d = EIN("WR2", [DIMS[0], DIMS[1]], bf16)
    WL3_d = EIN("WL3", [DIMS[1], DIMS[2]], bf16)
    WR3_d = EIN("WR3", [DIMS[1], DIMS[2]], bf16)
    xrfill_d = [EIN(f"xrfill{i}", [6, NBLK * DIMS[i - 1]], bf16) for i in (1, 2, 3)]
    sgnB_d = [EIN(f"sgnB{i}", [128, DIMS[i - 1]], bf16) for i in (1, 2, 3)]
    attrecip_d = [EIN(f"attrecip{i}", [128, DIMS[i - 1]]) for i in (2, 3)]
    biasRep_d = [EIN(f"biasRep{i}", [128, DIMS[i - 1]]) for i in (1, 2, 3)]
    iota_d = EIN("iota_row", [128, 128], bf16)
    ident_d = EIN("ident", [128, 128], bf16)
    batchloc_d = EIN("batchloc", [128, NBLK], bf16)
    g_rows_d = EIN("g_rows", [128, 1], i32)
    rcnt_d = EIN("rcnt", [128, 4])
    w4rep_d = EIN("w4rep", [128, 8])
    b4_d = EIN("b4v", [128, 1])

    out_d = nc.dram_tensor("out", [B, 1], f32, kind="ExternalOutput")

    # table/stage row width padded to 128 for layer 3 (dma_gather needs
    # 256-byte rows); cols 8:128 are never read
    TW = [None, DIMS[1], 128]
    tables = [None,
              nc.dram_tensor("table2", [NC * PADN, TW[1]], bf16,
                             addr_space="Shared"),
              nc.dram_tensor("table3", [NC * PADN, TW[2]], bf16,
                             addr_space="Shared")]
    stages = [nc.dram_tensor("stage2", [PADN, TW[1]], bf16),
              nc.dram_tensor("stage3", [PADN, TW[2]], bf16)]
    pool_full = nc.dram_tensor("pool_full", [POOLPAD, 8], f32)
    pool_red = nc.dram_tensor("pool_red", [B, 8], f32, addr_space="Shared")

    with tile.TileContext(nc) as tc:
        ctx = contextlib.ExitStack()
        with ctx:
            consts = ctx.enter_context(tc.tile_pool(name="consts", bufs=1))
            meta = ctx.enter_context(tc.tile_pool(name="meta", bufs=1))
            xrp = ctx.enter_context(tc.tile_pool(name="xrp", bufs=1))
            stp = ctx.enter_context(tc.tile_pool(name="stp", bufs=3))
            gp = ctx.enter_context(tc.tile_pool(name="gp", bufs=2))
            sp = ctx.enter_context(tc.tile_pool(name="sp", bufs=3))
            ep = ctx.enter_context(tc.tile_pool(name="ep", bufs=2))
            pst = ctx.enter_context(tc.tile_pool(name="psum_t", bufs=2, space="PSUM"))
            psb = ctx.enter_context(tc.tile_pool(name="psum_blk", bufs=2, space="PSUM"))
            pse = ctx.enter_context(tc.tile_pool(name="psum_epi", bufs=1, space="PSUM"))
            chp = ctx.enter_context(tc.tile_pool(name="chunk", bufs=2))

            def load_const(dram, shape, d=f32):
                t = consts.tile(list(shape), d, tag=dram.name + "_c")
                nc.sync.dma_start(t[:], dram[:])
                return t
            iota_t = load_const(iota_d, [128, 128], bf16)
            ident_t = load_const(ident_d, [128, 128], bf16)
            WR1f_t = load_const(WR1f_d, [5, DIMS[0]])
            Wbd_t = load_const(Wbd_d, [40, DIMS[0]], bf16)
            WL2_t = [consts.tile([128, DIMS[1]], bf16, tag=f"wl2_{k}", name=f"wl2_{k}")
                     for k in range(2)]
            WR2_t = [consts.tile([128, DIMS[1]], bf16, tag=f"wr2_{k}", name=f"wr2_{k}")
                     for k in range(2)]
            for k in range(2):
                nc.sync.dma_start(WL2_t[k][:], WL2_d[k * 128:(k + 1) * 128, :])
                nc.sync.dma_start(WR2_t[k][:], WR2_d[k * 128:(k + 1) * 128, :])
            WL3_t = load_const(WL3_d, [128, DIMS[2]], bf16)
            WR3_t = load_const(WR3_d, [128, DIMS[2]], bf16)
            sgnB_t = [load_const(sgnB_d[i], [128, DIMS[i]], bf16) for i in range(3)]
            attrecip_t = [None,
                          load_const(attrecip_d[0], [128, DIMS[1]]),
                          load_const(attrecip_d[1], [128, DIMS[2]])]
            biasRep_t = [load_const(biasRep_d[i], [128, DIMS[i]]) for i in range(3)]
            batchloc_t = load_const(batchloc_d, [128, NBLK], bf16)
            g_rows_t = load_const(g_rows_d, [128, 1], i32)
            rcnt_t = load_const(rcnt_d, [128, 4])
            w4rep_t = load_const(w4rep_d, [128, 8])
            b4_t = load_const(b4_d, [128, 1])
            idx_t = meta.tile([128, T * 8], i16)
            nc.sync.dma_start(idx_t[:], idx16_d[:])
            dst_t = meta.tile([128, T], bf16)
            nc.sync.dma_start(dst_t[:], dst_sb_d[:])
            extE_t = meta.tile([128, T, 5], bf16)
            nc.sync.dma_start(extE_t[:].rearrange("p t f -> p (t f)"), extE_d[:])

            xr_t = [xrp.tile([128, NBLK * DIMS[i]], bf16, tag=f"xr{i}", name=f"xr{i}")
                    for i in range(3)]
            # rows 122-127 per layer: [WL1f ; weaug1] for layer 1,
            # [zeros ; weaug] for layers 2/3
            for i in range(3):
                nc.sync.dma_start(xr_t[i][122:128, :], xrfill_d[i][:])

            # ---- preamble: own xr1 blocks (f32 math, bf16 out) ----
            CH = 16
            for ch in range(-(-NBLK // CH)):
                j0, j1 = ch * CH, min((ch + 1) * CH, NBLK)
                xchunk = chp.tile([5, CH * 128], f32, tag="xchunk")
                nc.sync.dma_start(xchunk[:, :(j1 - j0) * 128],
                                  xt6_own_d[:, j0 * 128:j1 * 128])
                for j in range(j1 - j0):
                    b = j0 + j
                    pt = pse.tile([128, DIMS[0]], f32, tag="epi_ps", space="PSUM")
                    nc.tensor.matmul(pt[:], lhsT=xchunk[:, j * 128:(j + 1) * 128],
                                     rhs=WR1f_t[:], start=True, stop=True)
                    D0 = DIMS[0]
                    nc.vector.tensor_copy(xr_t[0][0:122, b * D0:(b + 1) * D0],
                                          pt[0:122, :])

            # ---- layers ----
            pool_ps = psb.tile([128, 8], f32, tag="pool_ps", space="PSUM", bufs=1)
            for li in range(3):
                H, C = HEADS[li]
                D = DIMS[li]
                GB = GBS[li]
                W = 48 if li == 0 else WIDTHS[li]
                table = tables[li]
                is_last = li == 2

                if li > 0:
                    nc.gpsimd.collective_compute(
                        "AllGather", Alu.bypass,
                        replica_groups=[list(range(NC))],
                        ins=[stages[li - 1].ap().opt()],
                        outs=[table.ap().opt()],
                    )

                t0 = 0
                for b in range(NBLK):
                    nt = tiles_pb[b]
                    ka = kas[b]
                    pblk = psb.tile([128, W], f32, tag="blk_ps", space="PSUM")
                    sblk = stp.tile([128, MAXNT * 128], bf16, tag="st_blk", bufs=2)
                    nc.sync.dma_start(sblk[:, :nt * 128], st_blk_d[b, :, :nt * 128])
                    if li > 0:
                        gblk = gp.tile([128, MAXNT, TW[li]], bf16, tag="g_blk",
                                       name="g_blk", bufs=3)
                        if ka:
                            nc.gpsimd.dma_gather(
                                gblk[:, 0:ka, :], table[0:SPLIT16, :],
                                idx_t[:, t0 * 8:(t0 + ka) * 8], ka * 128,
                                ka * 128, TW[li], single_packet=False)
                        if nt > ka:
                            nc.gpsimd.dma_gather(
                                gblk[:, ka:nt, :], table[SPLIT16:NC * PADN, :],
                                idx_t[:, (t0 + ka) * 8:(t0 + nt) * 8],
                                (nt - ka) * 128, (nt - ka) * 128, TW[li],
                                single_packet=False)
                    # pass 1: per-tile pre-activations + per-edge scatter rows
                    # into block-sized buffers (keeps the PE MM1 stream dense)
                    ytb = gp.tile([128, MAXNT, max(W, 48)], bf16, tag="y_blk",
                                  name="y_blk", bufs=2)
                    smb = stp.tile([128, MAXNT * 128], bf16, tag="s_blk", bufs=2)
                    for g0 in range(0, nt, GB):
                        gs = min(GB, nt - g0)
                        ptile = pst.tile([128, GB, D], f32, tag="t_ps",
                                         name=f"t_ps{li}", space="PSUM")
                        if li > 0:
                            # wide gather-add FIRST (opens the psum group for
                            # the whole bank), then per-tile one-hot matmuls
                            # accumulate on top
                            nc.tensor.matmul(
                                ptile[:, 0:gs, 0:D], lhsT=ident_t[:],
                                rhs=gblk[:, g0:g0 + gs, 0:D],
                                start=True, stop=False, skip_group_check=True)
                        for i in range(gs):
                            nc.tensor.matmul(
                                ptile[:, i, 0:D],
                                lhsT=sblk[:, (g0 + i) * 128:(g0 + i + 1) * 128],
                                rhs=xr_t[li][:, b * D:(b + 1) * D],
                                start=(li == 0),
                                stop=(li == 0) or (i == gs - 1),
                                skip_group_check=(li > 0))
                        # logits = sum_c sgn * prelu(t_s, 0.2); ex = exp(logits)
                        u = sp.tile([128, GB * D], bf16, tag="u_t")
                        nc.scalar.activation(
                            u[:, :gs * D].rearrange("p (g d) -> p g d", g=gs),
                            ptile[:, 0:gs, 0:D], Act.Prelu, alpha=0.2)
                        v = sp.tile([128, GB * D], bf16, tag="v_t")
                        nc.vector.tensor_tensor(
                            out=v[:, :gs * D].rearrange("p (g d) -> p g d", g=gs),
                            in0=u[:, :gs * D].rearrange("p (g d) -> p g d", g=gs),
                            in1=sgnB_t[li][:].unsqueeze(1).to_broadcast([128, gs, D]),
                            op=Alu.mult)
                        lg = sp.tile([128, GB * H], f32, tag="lg")
                        nc.vector.tensor_reduce(
                            out=lg[:, :gs * H].rearrange("p (g h) -> p g h", g=gs),
                            in_=v[:, :gs * D].rearrange("p (g h c) -> p g h c",
                                                        g=gs, h=H),
                            axis=mybir.AxisListType.X, op=Alu.add)
                        yt = ytb[:, g0:g0 + gs, 0:W]
                        nc.scalar.activation(
                            yt[:, :, W - H:W], lg[:, :gs * H].rearrange(
                                "p (g h) -> p g h", g=gs), Act.Exp)
                        if li == 0:
                            nc.vector.tensor_tensor(
                                out=yt[:, :, 0:40].rearrange(
                                    "p g (h f) -> p g h f", h=8),
                                in0=yt[:, :, 40:48].unsqueeze(3)
                                    .to_broadcast([128, gs, 8, 5]),
                                in1=extE_t[:, t0 + g0:t0 + g0 + gs, :].unsqueeze(2)
                                    .to_broadcast([128, gs, 8, 5]),
                                op=Alu.mult)
                        else:
                            nc.vector.tensor_tensor(
                                out=yt[:, :, 0:D].rearrange(
                                    "p g (h c) -> p g h c", h=H),
                                in0=gblk[:, g0:g0 + gs, 0:D].rearrange(
                                    "p g (h c) -> p g h c", h=H),
                                in1=yt[:, :, D:W].unsqueeze(3)
                                    .to_broadcast([128, gs, H, C]),
                                op=Alu.mult)
                        nc.vector.tensor_tensor(
                            out=smb[:, g0 * 128:(g0 + gs) * 128].rearrange(
                                "p (g n) -> p g n", g=gs),
                            in0=dst_t[:, t0 + g0:t0 + g0 + gs].unsqueeze(2)
                                .to_broadcast([128, gs, 128]),
                            in1=iota_t[:].unsqueeze(1).to_broadcast([128, gs, 128]),
                            op=Alu.is_equal)
                    # pass 2: all scatter matmuls back-to-back
                    for t in range(nt):
                        nc.tensor.matmul(
                            pblk[:], lhsT=smb[:, t * 128:(t + 1) * 128],
                            rhs=ytb[:, t, 0:W],
                            start=(t == 0), stop=(t == nt - 1))
                    t0 += nt

                    # ---- block epilogue ----
                    den = sp.tile([128, H], f32, tag="den")
                    nc.vector.tensor_scalar_add(den[:], pblk[:, W - H:W], 1e-30)
                    rden = sp.tile([128, H], f32, tag="rden")
                    nc.vector.reciprocal(rden[:], den[:])
                    if li == 0:
                        # reconstruct numerator: (scat @ Wbd), attrecip refolded
                        scat_sb = ep.tile([128, 48], bf16, tag="scat_sb")
                        nc.vector.tensor_copy(scat_sb[:], pblk[:, 0:48])
                        tps = pse.tile([128, 128], bf16, tag="epi_ps", space="PSUM")
                        nc.tensor.transpose(tps[0:48, :], scat_sb[:], ident_t[:])
                        scatT = ep.tile([48, 128], bf16, tag="scatT")
                        nc.vector.tensor_copy(scatT[:], tps[0:48, :])
                        pnum = pse.tile([128, DIMS[0]], f32, tag="epi_ps",
                                        space="PSUM")
                        nc.tensor.matmul(pnum[:], lhsT=scatT[0:40, :], rhs=Wbd_t[:],
                                         start=True, stop=True)
                        hr = ep.tile([128, D], f32, tag="hr")
                        nc.vector.tensor_tensor(
                            out=hr[:].rearrange("p (h c) -> p h c", h=H),
                            in0=pnum[:].rearrange("p (h c) -> p h c", h=H),
                            in1=rden[:].unsqueeze(2).to_broadcast([128, H, C]),
                            op=Alu.mult)
                    else:
                        hr = ep.tile([128, D], f32, tag="hr")
                        nc.vector.tensor_tensor(
                            out=hr[:].rearrange("p (h c) -> p h c", h=H),
                            in0=pblk[:, 0:D].rearrange("p (h c) -> p h c", h=H),
                            in1=rden[:].unsqueeze(2).to_broadcast([128, H, C]),
                            op=Alu.mult)
                        nc.vector.tensor_tensor(out=hr[:], in0=hr[:],
                                                in1=attrecip_t[li][:], op=Alu.mult)
                    nc.vector.tensor_tensor(out=hr[:], in0=hr[:],
                                            in1=biasRep_t[li][:], op=Alu.add)
                    h = ep.tile([128, D], bf16, tag="h_blk")
                    nc.scalar.activation(h[:], hr[:], Act.Tanh)

                    if not is_last:
                        D2 = DIMS[li + 1]
                        WLn = [WL2_t[0], WL2_t[1]] if li == 0 else [WL3_t]
                        WRn = [WR2_t[0], WR2_t[1]] if li == 0 else [WR3_t]
                        nk = D // 128
                        hT = []
                        for k in range(nk):
                            tp = pse.tile([128, 128], bf16, tag="epi_ps",
                                          space="PSUM")
                            nc.tensor.transpose(tp[:], h[:, k * 128:(k + 1) * 128],
                                                ident_t[:])
                            hTk = ep.tile([128, 128], bf16, tag=f"hT{k}")
                            nc.vector.tensor_copy(hTk[:], tp[:])
                            hT.append(hTk)
                        pxl = pse.tile([128, D2], f32, tag="epi_ps", space="PSUM")
                        for k in range(nk):
                            nc.tensor.matmul(pxl[:], lhsT=hT[k][:], rhs=WLn[k][:],
                                             start=(k == 0), stop=(k == nk - 1))
                        xlout = ep.tile([128, D2], bf16, tag="xlout")
                        nc.vector.tensor_copy(xlout[:], pxl[:])
                        nc.sync.dma_start(
                            stages[li][b * 128:(b + 1) * 128, 0:D2], xlout[:])
                        pxr = pse.tile([128, D2], f32, tag="epi_ps", space="PSUM")
                        for k in range(nk):
                            nc.tensor.matmul(pxr[:], lhsT=hT[k][:], rhs=WRn[k][:],
                                             start=(k == 0), stop=(k == nk - 1))
                        nc.vector.tensor_copy(
                            xr_t[li + 1][0:122, b * D2:(b + 1) * D2], pxr[0:122, :])
                    else:
                        Sg = stp.tile([128, 128], bf16, tag="sg_tile")
                        nc.vector.tensor_tensor(
                            out=Sg[:],
                            in0=batchloc_t[:, b:b + 1].to_broadcast([128, 128]),
                            in1=iota_t[:], op=Alu.is_equal)
                        nc.tensor.matmul(pool_ps[:], lhsT=Sg[:], rhs=h[:],
                                         start=(b == 0), stop=(b == NBLK - 1))

            # ---- pooling + head ----
            pool_sb = ep.tile([128, 8], f32, tag="pool_sb")
            nc.vector.tensor_copy(pool_sb[:], pool_ps[:])
            zero8 = consts.tile([128, 8], f32, tag="zero8")
            nc.gpsimd.memset(zero8[:], 0.0)
            for i in range(POOLPAD // 128):
                nc.sync.dma_start(pool_full[i * 128:(i + 1) * 128, :], zero8[:])
            nc.gpsimd.indirect_dma_start(
                out=pool_full[:], out_offset=IOA(ap=g_rows_t[:, :1], axis=0),
                in_=pool_sb[:], in_offset=None)
            nc.gpsimd.collective_compute(
                "AllReduce", Alu.add, replica_groups=[list(range(NC))],
                ins=[pool_full.ap()[0:B, :].opt()], outs=[pool_red.ap().opt()])
            for i in range(B // 128):
                pt = ep.tile([128, 8], f32, tag="head_in")
                nc.sync.dma_start(pt[:], pool_red[i * 128:(i + 1) * 128, :])
                pw = ep.tile([128, 8], f32, tag="head_w")
                nc.vector.tensor_tensor(out=pw[:], in0=pt[:], in1=w4rep_t[:],
                                        op=Alu.mult)
                hred = ep.tile([128, 1], f32, tag="head_red")
                nc.vector.tensor_reduce(out=hred[:], in_=pw[:],
                                        axis=mybir.AxisListType.X, op=Alu.add)
                nc.vector.tensor_tensor(out=hred[:], in0=hred[:],
                                        in1=rcnt_t[:, i:i + 1], op=Alu.mult)
                nc.vector.tensor_tensor(out=hred[:], in0=hred[:], in1=b4_t[:],
                                        op=Alu.add)
                nc.sync.dma_start(out_d[i * 128:(i + 1) * 128, :], hred[:])

    nc.compile()
    return nc


def _get_program(inputs):
    pre = _host_preprocess(inputs["x"], inputs["edge_index"], inputs["edge_attr"],
                           inputs["batch"])
    key = (tuple(pre["tiles_pb"]), tuple(pre["kas"]))
    if key not in _CACHE:
        _CACHE[key] = _build_program(pre["tiles_pb"], pre["kas"], pre["T"])
    return _CACHE[key], pre


def _make_in_maps(inputs, pre):
    import ml_dtypes
    bf16 = ml_dtypes.bfloat16
    wts = _host_weights(inputs)
    xt6_own = _build_x_inputs(inputs["x"])
    iota = np.tile(np.arange(128, dtype=np.float32), (128, 1))
    ident = np.eye(128, dtype=np.float32)
    in_maps = []
    for c in range(NC):
        m = dict(
            st_blk=pre["st_blk"][c].astype(bf16),
            idx16=pre["idx16"][c],
            dst_sb=pre["dst_sb"][c].astype(bf16),
            extE=pre["extE_sb"][c].astype(bf16),
            xt6_own=xt6_own[c],
            WR1f=wts["WR1f"], Wbd=wts["Wbd"].astype(bf16),
            WL2=wts["WL2"].astype(bf16), WR2=wts["WR2"].astype(bf16),
            WL3=wts["WL3"].astype(bf16), WR3=wts["WR3"].astype(bf16),
            iota_row=iota.astype(bf16), ident=ident.astype(bf16),
            batchloc=pre["batchloc"][c].astype(bf16), g_rows=pre["g_rows"][c],
            rcnt=np.ascontiguousarray(pre["rcnt"].reshape(4, 128).T),
            w4rep=wts["w4rep"], b4v=np.full((128, 1), wts["b4"], np.float32),
        )
        for i in (1, 2, 3):
            m[f"xrfill{i}"] = wts[f"xrfill{i}"].astype(bf16)
            m[f"sgnB{i}"] = wts[f"sgnB{i}"].astype(bf16)
            m[f"biasRep{i}"] = wts[f"biasRep{i}"]
            if i > 1:
                m[f"attrecip{i}"] = wts[f"attrecip{i}"]
        in_maps.append(m)
    return in_maps


def kernel(**inputs):
    from concourse.bass_utils import run_bass_kernel_spmd
    nc, pre = _get_program(inputs)
    in_maps = _make_in_maps(inputs, pre)
    res = run_bass_kernel_spmd(nc, in_maps, core_ids=list(range(NC)))
    return np.asarray(res.results[0]["out"], np.float32)
